# revision 8
# baseline (speedup 1.0000x reference)
"""Trainium2 Bass kernel for nn_CT_loss (data-parallel over batch, 8 cores).

Math (R is a general 3x3 matrix, not orthogonal):
  u   = A P0 + b0          A = R diag(e), b0 = t - 0.5 R e      (per batch)
  c   = G P0 + g0          G = R^T A,     g0 = R^T b0
  v_x = A[:,1] Q0 + A[:,2] Q1 + h_x      h_x = t - .5(A[:,1]+A[:,2])
  v_y = A[:,0] Q2 + A[:,2] Q3 + h_y      h_y = t - .5(A[:,0]+A[:,2])
  v_z = A[:,0] Q4 + A[:,1] Q5 + h_z      h_z = t - .5(A[:,0]+A[:,1])
  s   = R^T t
  d_ai = s_a u_i - c_a v_ai ;  w_a = sum_i d_ai^2
  la_a = sqrt(w_a * m_a)            (mask in {0,1} folded before sqrt)
  loss = sum_a [sum(m_a) >= 3B] sum(la_a) / max(sum_a sum(m_a), 1)

Device layout per core: 8 batches; SBUF tiles [128, 1024] where partition
p = local_batch*16 + pixel_group, free = 1024 pixels. Per-batch scalars are
per-partition [128,1] columns of a constants tile. Free-dim sums come from
accum_out on the last op of each chain; host finishes the 128-row +
cross-core reduction (the "gather").
"""
import os
import sys

import numpy as np

for _p in ("/opt/trn_rl_repo",):
    if _p not in sys.path:
        sys.path.insert(0, _p)

import concourse.bass as bass
import concourse.bacc as bacc
import concourse.tile as tile
from concourse import mybir
from concourse.bass_utils import run_bass_kernel_spmd

from ml_dtypes import bfloat16

F32 = mybir.dt.float32
BF16 = mybir.dt.bfloat16
AF = mybir.ActivationFunctionType
OP = mybir.AluOpType

B, HW = 64, 128 * 128
NCORES, BPC, G, FD = 8, 8, 16, 1024  # 8 batches/core, 16 pixel groups of 1024
NCST = 36

# a -> (Acol1, Acol2, qchA, qchB):  v_a = A[:,c1]*Q[qA] + A[:,c2]*Q[qB] + h_a
QCH = {0: (1, 2, 0, 1), 1: (0, 2, 2, 3), 2: (0, 1, 4, 5)}

# constants tile column layout
CA = 0        # A row-major 9
CB0 = 9       # b0 3
CG = 12       # G row-major 9
CG0 = 21      # g0 3
CS = 24       # s 3
CH = 27       # h[a*3+i] 9

# ---- engine assignment knobs (tuned by profiling) ----
E_UC_HEAD = "act"      # affine head of u/c chains: act | vec | pool
E_V_HEAD = "vec"       # affine head of v chains
E_SQ = "act"           # squares: act | vec | pool
E_W = "pool"           # w = sq+sq+sq adds: vec | pool
E_WM = "pool"          # w*m: vec | pool
E_MSUM = "vec"         # mask sums: vec | pool
E_DIFF_TT = "vec"      # c*v products: vec | pool

_BUILT = None
LAST = None  # last BassKernelResults (for test harness)


def _affine(nc, eng, out, in_, sc, bi):
    """out = in_*sc + bi with per-partition scalars."""
    if eng == "act":
        nc.scalar.activation(out, in_, AF.Identity, bias=bi, scale=sc)
    elif eng == "vec":
        nc.vector.tensor_scalar(out, in_, sc, bi, op0=OP.mult, op1=OP.add)
    else:
        nc.gpsimd.tensor_scalar(out, in_, sc, bi, op0=OP.mult, op1=OP.add)


def _mul(nc, eng, out, a, b):
    (nc.vector if eng == "vec" else nc.gpsimd).tensor_mul(out, a, b)


def _add(nc, eng, out, a, b):
    (nc.vector if eng == "vec" else nc.gpsimd).tensor_add(out, a, b)


def _build_nc():
    nc = bacc.Bacc(None)
    p0 = nc.dram_tensor("p0", [BPC, G, 3, FD], BF16, kind="ExternalInput")
    q0 = nc.dram_tensor("q0", [BPC, G, 6, FD], BF16, kind="ExternalInput")
    mk = nc.dram_tensor("mk", [BPC, G, 3, FD], BF16, kind="ExternalInput")
    cst = nc.dram_tensor("cst", [128, NCST], F32, kind="ExternalInput")
    outp = nc.dram_tensor("out", [128, 6], F32, kind="ExternalOutput")

    with tile.TileContext(nc) as tc:
        with tc.tile_pool(name="main", bufs=1) as pool:
            cst_t = pool.tile([128, NCST], F32, tag="cst")
            nc.sync.dma_start(cst_t[:], cst[:])

            def cs(j):
                return cst_t[:, j:j + 1]

            p0_t = pool.tile([128, 3, FD], BF16, tag="p0")
            nc.sync.dma_start(p0_t[:], p0[:].rearrange("b g c f -> (b g) c f"))
            q0_t = pool.tile([128, 6, FD], BF16, tag="q0")
            nc.sync.dma_start(q0_t[:], q0[:].rearrange("b g c f -> (b g) c f"))
            mk_t = pool.tile([128, 3, FD], BF16, tag="mk")
            nc.sync.dma_start(mk_t[:], mk[:].rearrange("b g c f -> (b g) c f"))

            acc = pool.tile([128, 6], F32, tag="acc")

            X = [p0_t[:, j, :] for j in range(3)]
            Q = [q0_t[:, j, :] for j in range(6)]
            MSK = [mk_t[:, a, :] for a in range(3)]

            # u_i = A_i0 X0 + A_i1 X1 + A_i2 X2 + b0_i ; c_a likewise with G,g0
            u, c = [], []
            for i in range(3):
                t = pool.tile([128, FD], BF16, tag=f"u{i}")
                _affine(nc, E_UC_HEAD, t, X[2], cs(CA + 3 * i + 2), cs(CB0 + i))
                nc.vector.scalar_tensor_tensor(t, X[1], cs(CA + 3 * i + 1), t,
                                               op0=OP.mult, op1=OP.add)
                nc.vector.scalar_tensor_tensor(t, X[0], cs(CA + 3 * i + 0), t,
                                               op0=OP.mult, op1=OP.add)
                u.append(t)
            for a in range(3):
                t = pool.tile([128, FD], BF16, tag=f"c{a}")
                _affine(nc, E_UC_HEAD, t, X[2], cs(CG + 3 * a + 2), cs(CG0 + a))
                nc.vector.scalar_tensor_tensor(t, X[1], cs(CG + 3 * a + 1), t,
                                               op0=OP.mult, op1=OP.add)
                nc.vector.scalar_tensor_tensor(t, X[0], cs(CG + 3 * a + 0), t,
                                               op0=OP.mult, op1=OP.add)
                c.append(t)

            scr = pool.tile([128, FD], BF16, tag="scr")  # Msum dummy out
            for a in range(3):
                c1, c2, qA, qB = QCH[a]
                sq = []
                for i in range(3):
                    v = pool.tile([128, FD], BF16, tag=f"v{a}{i}")
                    _affine(nc, E_V_HEAD, v, Q[qB], cs(CA + 3 * i + c2),
                            cs(CH + 3 * a + i))
                    nc.vector.scalar_tensor_tensor(v, Q[qA], cs(CA + 3 * i + c1),
                                                   v, op0=OP.mult, op1=OP.add)
                    # t_ai = c_a * v_ai ; d_ai = u_i*s_a - t_ai
                    _mul(nc, E_DIFF_TT, v, c[a], v)
                    d = pool.tile([128, FD], BF16, tag=f"d{a}{i}")
                    nc.vector.scalar_tensor_tensor(d, u[i], cs(CS + a), v,
                                                   op0=OP.mult, op1=OP.subtract)
                    s = pool.tile([128, FD], BF16, tag=f"sq{a}{i}")
                    if E_SQ == "act":
                        nc.scalar.activation(s, d, AF.Square)
                    else:
                        _mul(nc, E_SQ, s, d, d)
                    sq.append(s)
                w = pool.tile([128, FD], BF16, tag=f"w{a}")
                _add(nc, E_W, w, sq[0], sq[1])
                _add(nc, E_W, w, w, sq[2])
                _mul(nc, E_WM, w, w, MSK[a])
                la = pool.tile([128, FD], BF16, tag=f"la{a}")
                nc.scalar.activation(la, w, AF.Sqrt,
                                     accum_out=acc[:, a:a + 1])
                eng = nc.vector if E_MSUM == "vec" else nc.gpsimd
                eng.tensor_scalar(scr, MSK[a], 1.0, None, op0=OP.mult,
                                  op1=OP.add, accum_out=acc[:, 3 + a:4 + a])

            nc.sync.dma_start(outp[:], acc[:])

    nc.compile()
    return nc


def get_nc():
    global _BUILT
    if _BUILT is None:
        _BUILT = _build_nc()
    return _BUILT


def host_constants(R, T, E):
    """[B, NCST] fp32 per-batch constants (computed in fp64)."""
    Bn = R.shape[0]
    out = np.zeros((Bn, NCST), np.float64)
    for b in range(Bn):
        Rb = R[b].astype(np.float64)
        tb = T[b].astype(np.float64)
        eb = E[b].astype(np.float64)
        A = Rb * eb[None, :]
        b0 = tb - 0.5 * (Rb @ eb)
        Gm = Rb.T @ A
        g0 = Rb.T @ b0
        s = Rb.T @ tb
        out[b, CA:CA + 9] = A.reshape(-1)
        out[b, CB0:CB0 + 3] = b0
        out[b, CG:CG + 9] = Gm.reshape(-1)
        out[b, CG0:CG0 + 3] = g0
        out[b, CS:CS + 3] = s
        for a, (c1, c2, _, _) in QCH.items():
            out[b, CH + 3 * a:CH + 3 * a + 3] = tb - 0.5 * (A[:, c1] + A[:, c2])
    return out.astype(np.float32)


def make_in_maps(P0, Q0, M, cst):
    in_maps = []
    for k in range(NCORES):
        sl = slice(k * BPC, (k + 1) * BPC)
        in_maps.append({
            "p0": P0[sl].reshape(BPC, 3, G, FD).transpose(0, 2, 1, 3).astype(bfloat16),
            "q0": Q0[sl].reshape(BPC, 6, G, FD).transpose(0, 2, 1, 3).astype(bfloat16),
            "mk": M[sl].reshape(BPC, 3, G, FD).transpose(0, 2, 1, 3).astype(bfloat16),
            "cst": np.ascontiguousarray(np.repeat(cst[sl], G, axis=0)),
        })
    return in_maps


def kernel(pred_rots, pred_P0, pred_Q0, gt_occmask, roi_extent, pred_transes):
    global LAST
    R = np.asarray(pred_rots, np.float32)
    P0 = np.asarray(pred_P0, np.float32)
    Q0 = np.asarray(pred_Q0, np.float32)
    M = np.asarray(gt_occmask, np.float32)
    E = np.asarray(roi_extent, np.float32)
    T = np.asarray(pred_transes, np.float32)

    nc = get_nc()
    cst = host_constants(R, T, E)
    in_maps = make_in_maps(P0, Q0, M, cst)
    trace = os.environ.get("KERNEL_TRACE", "0") == "1"
    LAST = run_bass_kernel_spmd(nc, in_maps, core_ids=list(range(NCORES)),
                                trace=trace)
    S_a = np.zeros(3, np.float64)
    M_a = np.zeros(3, np.float64)
    for r in LAST.results:
        o = r["out"].astype(np.float64)
        S_a += o[:, 0:3].sum(axis=0)
        M_a += o[:, 3:6].sum(axis=0)
    loss = sum(0.0 if M_a[a] < 3 * B else S_a[a] for a in range(3))
    total = max(M_a.sum(), 1.0)
    return np.asarray(np.float32(loss / total))
